# revision 8
# baseline (speedup 1.0000x reference)
import sys

if "/opt/trn_rl_repo" not in sys.path:
    sys.path.insert(0, "/opt/trn_rl_repo")

import numpy as np

# Problem: y = LeakyReLU((conv2d(x, w, VALID) + bias) / 2, slope=0.01)
#   x: (32, 128, 130, 130) f32, w: (256, 128, 3, 3) f32, b: (256,) f32
#   y: (32, 256, 128, 128) f32
# Sharding: data-parallel over batch, 4 images per core on 8 cores.
# Per core: conv as implicit GEMM in fp32r (1 cycle/row on the PE, ~1.5e-4
# rel err) — for each output tile of 4 rows x 128 cols, accumulate 9 matmuls
# (one per 3x3 tap) of [K=128(C_in), M=128(C_out)] x [K=128, N=512] into one
# PSUM bank, then a single fused ACT epilogue
# Prelu(psum*0.5 + 0.5*bias, alpha=0.01) straight out of PSUM.
# x streams in row-chunks per image (the first chunk is small so the PE
# starts early); weights load as two j-halves right after the first x chunk,
# all on the sync (HWDGE) queue whose transfers run on parallel HW queues.

N_CORES = 8
IMGS_PER_CORE = 4
C_IN = 128
C_OUT = 256
H_IN = 130
W_IN = 130
H_OUT = 128
W_OUT = 128
ROWS_PER_TILE = 4            # output rows per matmul tile -> N = 4*128 = 512
N_TILE = ROWS_PER_TILE * W_OUT
DIVISOR = 2.0
SLOPE = 0.01

# (start_out_row, n_out_rows) chunk schedules; image 0 front-loads a small
# chunk so the first matmuls start as early as possible.
FIRST_IMG_CHUNKS = [(0, 8), (8, 40), (48, 40), (88, 40)]
OTHER_IMG_CHUNKS = [(0, 32), (32, 32), (64, 32), (96, 32)]
MAX_CHUNK_IN_ROWS = max(r for _, r in FIRST_IMG_CHUNKS + OTHER_IMG_CHUNKS) + 2
# SBUF x row stride. (Aligned-stride/offset variants and on-chip shifted
# copies were measured: any per-MM AP-alignment gain is eaten by the extra
# SBUF traffic; plain rows are fastest overall.)
ROW_STRIDE = W_IN

_CACHE = {}


def _build():
    import concourse.tile as tile
    import concourse.mybir as mybir
    from concourse import bacc

    F32 = mybir.dt.float32
    F32R = mybir.dt.float32r

    nc = bacc.Bacc(
        "TRN2",
        target_bir_lowering=False,
        debug=False,
        enable_asserts=True,
        num_devices=N_CORES,
    )

    x_d = nc.dram_tensor(
        "x", [IMGS_PER_CORE * C_IN, H_IN * W_IN], F32R, kind="ExternalInput"
    ).ap()
    # w free layout: j*1152 + ki*128 + co_lo   (j = cout tile, ki = 3x3 tap)
    w_d = nc.dram_tensor("w", [C_IN, 9 * C_OUT], F32R, kind="ExternalInput").ap()
    b_d = nc.dram_tensor("b", [C_OUT // 2, 2], F32, kind="ExternalInput").ap()
    y_d = nc.dram_tensor(
        "y", [IMGS_PER_CORE * C_OUT, H_OUT * W_OUT], F32, kind="ExternalOutput"
    ).ap()

    with tile.TileContext(nc) as tc:
        with (
            tc.tile_pool(name="const", bufs=1) as const_pool,
            tc.tile_pool(name="xbuf", bufs=4) as x_pool,
            tc.tile_pool(name="psum", bufs=8, space="PSUM") as psum_pool,
            tc.tile_pool(name="obuf", bufs=8) as out_pool,
        ):
            w_sb = const_pool.tile([C_IN, 9 * C_OUT], F32R)
            b_sb = const_pool.tile([C_OUT // 2, 2], F32)
            consts_loaded = False

            for n in range(IMGS_PER_CORE):
                chunks = FIRST_IMG_CHUNKS if n == 0 else OTHER_IMG_CHUNKS
                for row0, nrows in chunks:
                    in_rows = nrows + 2
                    xc = x_pool.tile([C_IN, MAX_CHUNK_IN_ROWS * ROW_STRIDE], F32R)
                    xv = xc[:, : in_rows * ROW_STRIDE].rearrange(
                        "p (h w) -> p h w", h=in_rows
                    )
                    nc.sync.dma_start(
                        xv[:, :, 0:W_IN],
                        x_d[
                            n * C_IN : (n + 1) * C_IN,
                            row0 * W_IN : (row0 + in_rows) * W_IN,
                        ].rearrange("p (h w) -> p h w", h=in_rows),
                    )
                    if not consts_loaded:
                        # issue right after the first (small) x chunk so the
                        # HW DMA queues run them all in parallel
                        consts_loaded = True
                        for j in range(2):
                            nc.sync.dma_start(
                                w_sb[:, j * 1152 : (j + 1) * 1152],
                                w_d[:, j * 1152 : (j + 1) * 1152],
                            )
                        nc.sync.dma_start(b_sb[:], b_d[:])
                    for gl in range(nrows // ROWS_PER_TILE):
                        g = row0 // ROWS_PER_TILE + gl
                        for j in range(2):  # cout tile
                            ps = psum_pool.tile([128, N_TILE], F32)
                            for ki in range(9):
                                kh, kw = divmod(ki, 3)
                                r0 = gl * ROWS_PER_TILE + kh
                                rhs = xv[:, r0 : r0 + ROWS_PER_TILE, kw : kw + W_OUT]
                                nc.tensor.matmul(
                                    ps[:],
                                    w_sb[
                                        :,
                                        j * 1152 + ki * 128 : j * 1152 + ki * 128 + 128,
                                    ],
                                    rhs,
                                    start=(ki == 0),
                                    stop=(ki == 8),
                                )
                            ot = out_pool.tile([128, N_TILE], F32)
                            nc.scalar.activation(
                                ot[:],
                                ps[:],
                                mybir.ActivationFunctionType.Prelu,
                                bias=b_sb[:, j : j + 1],
                                scale=1.0 / DIVISOR,
                                alpha=SLOPE,
                            )
                            nc.sync.dma_start(
                                y_d[
                                    n * C_OUT + j * 128 : n * C_OUT + (j + 1) * 128,
                                    g * N_TILE : (g + 1) * N_TILE,
                                ],
                                ot[:],
                            )

    nc.compile()
    return nc


# Results of the last hardware run (for test.py to pull profiling info from).
LAST_RESULT = None


def kernel(x, weight, bias):
    from concourse.bass_utils import run_bass_kernel_spmd

    global LAST_RESULT

    if "nc" not in _CACHE:
        _CACHE["nc"] = _build()
    nc = _CACHE["nc"]

    x = np.ascontiguousarray(x, dtype=np.float32)
    # [co, ci, kh, kw] -> [ci, j, ki, co_lo] -> [128, 2304]
    wt = np.ascontiguousarray(
        weight.astype(np.float32)
        .transpose(1, 2, 3, 0)
        .reshape(C_IN, 9, 2, 128)
        .transpose(0, 2, 1, 3)
    ).reshape(C_IN, 9 * C_OUT)
    # bias*0.5 as [128, 2]: column j = cout tile j
    bh = np.ascontiguousarray(
        (bias.astype(np.float32) / DIVISOR).reshape(2, 128).T
    )

    in_maps = []
    for c in range(N_CORES):
        xs = x[c * IMGS_PER_CORE : (c + 1) * IMGS_PER_CORE].reshape(
            IMGS_PER_CORE * C_IN, H_IN * W_IN
        )
        in_maps.append({"x": xs, "w": wt, "b": bh})

    res = run_bass_kernel_spmd(nc, in_maps, core_ids=list(range(N_CORES)))
    LAST_RESULT = res
    out = np.concatenate(
        [
            r["y"].reshape(IMGS_PER_CORE, C_OUT, H_OUT, W_OUT)
            for r in res.results
        ],
        axis=0,
    )
    return out
